# revision 10
# baseline (speedup 1.0000x reference)
"""Grouped-query attention (B=2,T=2048,D=2048, 4 groups x 4 heads x 128d) on 8 trn2 cores.

Sharding: core = (batch b, group g); b = core//4, g = core%4 (data parallel x tensor
parallel). Each core computes its group's QKV projections, QK-rmsnorm+rope, causal
flash-style attention, and a partial output projection o_g @ wo_g; the host sums the
4 per-group partials per batch (row-sharded wo all-reduce done on host at gather).

Device layout notes (per core):
  xt   [D, T] bf16 = x[b].T  -> projections produce qT/kT/vT [n, T] with head-dim on
       partitions, which feeds QK^T directly (scores transposed: [j, i], softmax sums
       over partitions via ones-matmul, PV uses v natural as lhsT).
  All matmuls bf16 with f32 PSUM accumulation. Softmax without max-subtraction:
  |scores| <= sqrt(128) by Cauchy-Schwarz after rmsnorm, so exp is safe in f32.
"""

import sys
from contextlib import ExitStack

for _p in ("/opt/trn_rl_repo", "/opt/pypackages"):
    if _p not in sys.path:
        sys.path.insert(0, _p)

import numpy as np
import ml_dtypes

import concourse.bass as bass
import concourse.mybir as mybir
import concourse.tile as tile
from concourse import bacc
from concourse.bass_utils import run_bass_kernel_spmd

bf16 = ml_dtypes.bfloat16
BF = mybir.dt.bfloat16
F32 = mybir.dt.float32
AF = mybir.ActivationFunctionType

B, T, D = 2, 2048, 2048
HD, H, G = 128, 4, 4
KC = D // 128          # 16 contraction chunks
TB = T // 128          # 16 t blocks
IC = T // 512          # 4 i chunks
EPS = 1e-6
MULT2 = float(HD) ** -0.5   # mult^2 folded into q gains

_NC_CACHE = {}


def _bcast(ap, p=128):
    """Partition-broadcast AP: [1, N] row -> [p, N] (step-0 partition dim)."""
    return bass.AP(tensor=ap.tensor, offset=ap.offset, ap=[[0, p]] + [list(a) for a in ap.ap[1:]])


def _build_nc():
    nc = bacc.Bacc(None)

    xt_d = nc.declare_dram_parameter("xt", [D, T], BF, isOutput=False)
    wq_d = nc.declare_dram_parameter("wq", [D, H * HD], BF, isOutput=False)
    wk_d = nc.declare_dram_parameter("wk", [D, HD], BF, isOutput=False)
    wv_d = nc.declare_dram_parameter("wv", [D, HD], BF, isOutput=False)
    wo_d = nc.declare_dram_parameter("wo", [H * HD, D], BF, isOutput=False)
    gqs_d = nc.declare_dram_parameter("gqs", [HD, H], F32, isOutput=False)
    gks_d = nc.declare_dram_parameter("gks", [HD, 1], F32, isOutput=False)
    cos_d = nc.declare_dram_parameter("cosf", [HD, T], BF, isOutput=False)
    sin_d = nc.declare_dram_parameter("sins", [HD, T], BF, isOutput=False)
    msk_d = nc.declare_dram_parameter("mask", [128, 128], F32, isOutput=False)
    idn_d = nc.declare_dram_parameter("ident", [128, 128], BF, isOutput=False)
    out_d = nc.declare_dram_parameter("out", [T, D], F32, isOutput=True)

    with tile.TileContext(nc) as tc:
        with ExitStack() as outer:
            persist = outer.enter_context(tc.tile_pool(name="persist", bufs=1))
            qhat = [persist.tile([128, T], BF, tag=f"qhat{h}", name=f"qhat{h}") for h in range(H)]
            khat = persist.tile([128, T], BF, tag="khat", name="khat")
            vnat = persist.tile([128, T], BF, tag="vnat", name="vnat")  # [j-local, tb*128+d]
            gqs = persist.tile([HD, H], F32, tag="gqs", name="gqs")
            gks = persist.tile([HD, 1], F32, tag="gks", name="gks")
            ones_bf = persist.tile([128, 1], BF, tag="ones", name="ones")
            eps_t = persist.tile([1, 1], F32, tag="eps", name="eps")

            nc.sync.dma_start(out=gqs, in_=gqs_d[:, :])
            nc.sync.dma_start(out=gks, in_=gks_d[:, :])
            nc.vector.memset(ones_bf, 1.0)
            nc.vector.memset(eps_t, EPS)

            # ---------------- Phase 1: projections + rmsnorm + rope ----------------
            with ExitStack() as s1:
                xt_p = s1.enter_context(tc.tile_pool(name="xt", bufs=1))
                w_p = s1.enter_context(tc.tile_pool(name="w", bufs=1))
                tmp_p = s1.enter_context(tc.tile_pool(name="tmp", bufs=1))
                row_p = s1.enter_context(tc.tile_pool(name="rows", bufs=1))
                dram_p = s1.enter_context(tc.tile_pool(name="dramb", bufs=2, space="DRAM"))
                ps_proj = s1.enter_context(tc.tile_pool(name="ps_proj", bufs=3, space="PSUM"))
                ps_row = s1.enter_context(tc.tile_pool(name="ps_row", bufs=2, space="PSUM"))
                ps_tp = s1.enter_context(tc.tile_pool(name="ps_tp", bufs=2, space="PSUM"))

                xt = []
                for kc in range(KC):
                    t_ = xt_p.tile([128, T], BF, tag=f"xt{kc}", name=f"xt{kc}")
                    nc.sync.dma_start(out=t_, in_=xt_d[kc * 128:(kc + 1) * 128, :])
                    xt.append(t_)
                wq_t, wk_t, wv_t = [], [], []
                for kc in range(KC):
                    a = w_p.tile([128, H * HD], BF, tag=f"wq{kc}", name=f"wq{kc}")
                    nc.sync.dma_start(out=a, in_=wq_d[kc * 128:(kc + 1) * 128, :])
                    wq_t.append(a)
                    b_ = w_p.tile([128, HD], BF, tag=f"wk{kc}", name=f"wk{kc}")
                    nc.sync.dma_start(out=b_, in_=wk_d[kc * 128:(kc + 1) * 128, :])
                    wk_t.append(b_)
                    c = w_p.tile([128, HD], BF, tag=f"wv{kc}", name=f"wv{kc}")
                    nc.sync.dma_start(out=c, in_=wv_d[kc * 128:(kc + 1) * 128, :])
                    wv_t.append(c)
                cosf = w_p.tile([HD, T], BF, tag="cosf", name="cosf")
                sins = w_p.tile([HD, T], BF, tag="sins", name="sins")
                ident = w_p.tile([128, 128], BF, tag="ident", name="ident")
                nc.sync.dma_start(out=cosf, in_=cos_d[:, :])
                nc.sync.dma_start(out=sins, in_=sin_d[:, :])
                nc.sync.dma_start(out=ident, in_=idn_d[:, :])

                def project(lhsT_of, raw_out):
                    """raw_out[:, tf*512:+512] = sum_kc lhsT(kc).T @ xt[kc][:, tf]."""
                    for tf in range(4):
                        ps = ps_proj.tile([128, 512], F32, tag="ps_proj", name="ps_proj")
                        for kc in range(KC):
                            nc.tensor.matmul(ps, lhsT_of(kc), xt[kc][:, tf * 512:(tf + 1) * 512],
                                             start=(kc == 0), stop=(kc == KC - 1))
                        nc.vector.tensor_copy(out=raw_out[:, tf * 512:(tf + 1) * 512], in_=ps)

                def norm_rope(raw, gain_col, hat_out):
                    """hat_out (bf16) = rope(gain*raw) * rsqrt(mean(raw^2)+eps), per column t."""
                    sq = tmp_p.tile([128, T], BF, tag="sq", name="sq")
                    nc.vector.tensor_mul(sq, raw, raw)
                    srow = row_p.tile([1, T], F32, tag="srow", name="srow", bufs=2)
                    for tf in range(4):
                        pr = ps_row.tile([1, 512], F32, tag="ps_row", name="ps_row")
                        nc.tensor.matmul(pr, ones_bf, sq[:, tf * 512:(tf + 1) * 512],
                                         start=True, stop=True)
                        nc.vector.tensor_copy(out=srow[:, tf * 512:(tf + 1) * 512], in_=pr)
                    # srow = sqrt(srow/HD + eps); then in-place rinv = 1/srow
                    nc.scalar.activation(out=srow, in_=srow, func=AF.Sqrt,
                                         bias=eps_t[:, 0:1], scale=1.0 / HD)
                    nc.vector.reciprocal_approx_fast(out=srow, in_=srow)
                    srow_d = dram_p.tile([1, T], F32, tag="srow_d", name="srow_d")
                    nc.sync.dma_start(out=srow_d, in_=srow)
                    rb = tmp_p.tile([128, T], F32, tag="rb", name="rb", bufs=2)
                    nc.sync.dma_start(out=rb, in_=_bcast(srow_d))
                    # gain (per-partition scalar, bf16 out), then rotate-half rope in bf16
                    gt = tmp_p.tile([128, T], BF, tag="gt", name="gt")
                    nc.vector.tensor_scalar_mul(gt, raw, gain_col)
                    sw = tmp_p.tile([128, T], BF, tag="sw", name="sw")
                    nc.sync.dma_start(out=sw[0:64, :], in_=gt[64:128, :])
                    nc.sync.dma_start(out=sw[64:128, :], in_=gt[0:64, :])
                    t1 = tmp_p.tile([128, T], BF, tag="t1", name="t1")
                    nc.vector.tensor_mul(t1, gt, cosf)
                    nc.vector.tensor_mul(sw, sw, sins)
                    nc.vector.tensor_add(t1, t1, sw)
                    nc.vector.tensor_mul(hat_out, t1, rb)

                # K
                kraw = tmp_p.tile([128, T], F32, tag="raw", name="kraw", bufs=2)
                project(lambda kc: wk_t[kc], kraw)
                norm_rope(kraw, gks[:, 0:1], khat)
                # V (vT then 128x128 transposes to natural layout)
                vtr = tmp_p.tile([128, T], BF, tag="vtr", name="vtr")
                project(lambda kc: wv_t[kc], vtr)
                for tb in range(TB):
                    pt = ps_tp.tile([128, 128], BF, tag="ps_tp", name="ps_tp")
                    nc.tensor.transpose(pt, vtr[:, tb * 128:(tb + 1) * 128], ident)
                    nc.vector.tensor_copy(out=vnat[:, tb * 128:(tb + 1) * 128], in_=pt)
                # Q heads
                for h in range(H):
                    qraw = tmp_p.tile([128, T], F32, tag="raw", name=f"qraw{h}", bufs=2)
                    project(lambda kc: wq_t[kc][:, h * 128:(h + 1) * 128], qraw)
                    norm_rope(qraw, gqs[:, h:h + 1], qhat[h])

            # ---------------- Phases 2+3 share the attention-output tiles ----------
            s23 = outer.enter_context(ExitStack())
            o_p = s23.enter_context(tc.tile_pool(name="op", bufs=1))
            oT = [o_p.tile([128, T], BF, tag=f"oT{h}", name=f"oT{h}") for h in range(H)]

            # ---------------- Phase 2: causal attention ----------------
            with ExitStack() as s2:
                mask_p = s2.enter_context(tc.tile_pool(name="maskp", bufs=1))
                p_p = s2.enter_context(tc.tile_pool(name="pexp", bufs=4))
                dn_p = s2.enter_context(tc.tile_pool(name="dn", bufs=2))
                db_p = s2.enter_context(tc.tile_pool(name="dnb", bufs=2))
                dram2_p = s2.enter_context(tc.tile_pool(name="dramb2", bufs=2, space="DRAM"))
                ps_s = s2.enter_context(tc.tile_pool(name="ps_s", bufs=3, space="PSUM"))
                ps_o = s2.enter_context(tc.tile_pool(name="ps_o", bufs=2, space="PSUM"))
                ps_d = s2.enter_context(tc.tile_pool(name="ps_d", bufs=2, space="PSUM"))

                mask = mask_p.tile([128, 128], F32, tag="mask", name="mask")
                nc.sync.dma_start(out=mask, in_=msk_d[:, :])

                for h in range(H):
                    for ic in range(IC):
                        i0 = ic * 512
                        jb_max = 4 * ic + 3
                        po = ps_o.tile([128, 512], F32, tag="ps_o", name="ps_o")
                        pd = ps_d.tile([1, 512], F32, tag="ps_d", name="ps_d")
                        for jb in range(jb_max + 1):
                            off = max(0, 128 * (jb - 4 * ic))
                            ps = ps_s.tile([128, 512], F32, tag="ps_s", name="ps_s")
                            nc.tensor.matmul(ps[:, off:], khat[:, jb * 128:(jb + 1) * 128],
                                             qhat[h][:, i0 + off:i0 + 512],
                                             start=True, stop=True)
                            if jb >= 4 * ic:
                                nc.vector.tensor_add(ps[:, off:off + 128],
                                                     ps[:, off:off + 128], mask)
                            p = p_p.tile([128, 512], BF, tag="p", name="p")
                            nc.scalar.activation(out=p[:, off:], in_=ps[:, off:], func=AF.Exp)
                            nc.tensor.matmul(po[:, off:], vnat[:, jb * 128:(jb + 1) * 128],
                                             p[:, off:], start=(jb == 0), stop=(jb == jb_max))
                            nc.tensor.matmul(pd[:, off:], ones_bf, p[:, off:],
                                             start=(jb == 0), stop=(jb == jb_max))
                        drow = dn_p.tile([1, 512], F32, tag="drow", name="drow")
                        nc.vector.tensor_copy(out=drow, in_=pd)
                        nc.vector.reciprocal_approx_fast(out=drow, in_=drow)
                        drow_d = dram2_p.tile([1, 512], F32, tag="drow_d", name="drow_d")
                        nc.sync.dma_start(out=drow_d, in_=drow)
                        db = db_p.tile([128, 512], F32, tag="db", name="db")
                        nc.sync.dma_start(out=db, in_=_bcast(drow_d))
                        nc.vector.tensor_mul(oT[h][:, i0:i0 + 512], po, db)

            # ---------------- Phase 3: output projection (partial over this group) ----
            with ExitStack() as s3:
                wo_p = s3.enter_context(tc.tile_pool(name="wo", bufs=1))
                ost_p = s3.enter_context(tc.tile_pool(name="ost", bufs=4))
                ps_out = s3.enter_context(tc.tile_pool(name="ps_out", bufs=4, space="PSUM"))

                wo_t = []
                for h in range(H):
                    w_ = wo_p.tile([128, D], BF, tag=f"wo{h}", name=f"wo{h}")
                    nc.sync.dma_start(out=w_, in_=wo_d[h * 128:(h + 1) * 128, :])
                    wo_t.append(w_)
                for tb in range(TB):
                    for oc in range(4):
                        ps = ps_out.tile([128, 512], F32, tag="ps_out", name="ps_out")
                        for h in range(H):
                            nc.tensor.matmul(ps, oT[h][:, tb * 128:(tb + 1) * 128],
                                             wo_t[h][:, oc * 512:(oc + 1) * 512],
                                             start=(h == 0), stop=(h == H - 1))
                        ost = ost_p.tile([128, 512], F32, tag="ost", name="ost")
                        nc.vector.tensor_copy(out=ost, in_=ps)
                        nc.sync.dma_start(out=out_d[tb * 128:(tb + 1) * 128,
                                                    oc * 512:(oc + 1) * 512], in_=ost)
    nc.finalize()
    return nc


def _rope_tables():
    d = np.arange(64, dtype=np.float64)
    ang = 10000.0 ** (-d / 64.0)
    pos = np.arange(T, dtype=np.float64)
    rad = pos[None, :] * ang[:, None]          # [64, T]
    cos, sin = np.cos(rad), np.sin(rad)
    cosF = np.concatenate([cos, cos], 0).astype(bf16)
    sinS = np.concatenate([-sin, sin], 0).astype(bf16)
    return np.ascontiguousarray(cosF), np.ascontiguousarray(sinS)


def _in_maps(x, wq, wk, wv, wo, gq, gk):
    cosF, sinS = _rope_tables()
    mask = np.ascontiguousarray(np.triu(np.full((128, 128), -1e9, np.float32), 1).T)
    ident = np.eye(128, dtype=bf16)
    maps = []
    for core in range(8):
        b, g = core // 4, core % 4
        maps.append({
            "xt": np.ascontiguousarray(x[b].T).astype(bf16),
            "wq": np.ascontiguousarray(wq[:, g * 512:(g + 1) * 512]).astype(bf16),
            "wk": np.ascontiguousarray(wk[:, g * 128:(g + 1) * 128]).astype(bf16),
            "wv": np.ascontiguousarray(wv[:, g * 128:(g + 1) * 128]).astype(bf16),
            "wo": np.ascontiguousarray(wo[g * 512:(g + 1) * 512, :]).astype(bf16),
            "gqs": np.ascontiguousarray((gq[g].T * MULT2).astype(np.float32)),
            "gks": np.ascontiguousarray(gk[g].astype(np.float32).reshape(HD, 1)),
            "cosf": cosF, "sins": sinS, "mask": mask, "ident": ident,
        })
    return maps


def _get_nc():
    if "nc" not in _NC_CACHE:
        _NC_CACHE["nc"] = _build_nc()
    return _NC_CACHE["nc"]


def _run(inputs, trace=False, trace_kwargs=None, tmpdir=None):
    nc = _get_nc()
    maps = _in_maps(inputs["x"], inputs["wq"], inputs["wk"], inputs["wv"],
                    inputs["wo"], inputs["gq"], inputs["gk"])
    res = run_bass_kernel_spmd(nc, maps, core_ids=list(range(8)), trace=trace,
                               tmpdir=tmpdir, **(trace_kwargs or {}))
    out = np.zeros((B, T, D), np.float32)
    for core in range(8):
        out[core // 4] += res.results[core]["out"]
    return out, res


def kernel(**inputs):
    inputs = {k: np.asarray(v) for k, v in inputs.items()}
    out, _ = _run(inputs, trace=False)
    return out


# revision 21
# speedup vs baseline: 1.1451x; 1.1451x over previous
"""Grouped-query attention (B=2,T=2048,D=2048, 4 groups x 4 heads x 128d) on 8 trn2 cores.

Sharding: core = (batch b, group g); b = core//4, g = core%4 (data parallel x tensor
parallel). Each core computes its group's QKV projections, QK-rmsnorm+rope, causal
flash-style attention, and a partial output projection o_g @ wo_g; the host sums the
4 per-group partials per batch (row-sharded wo all-reduce done on host at gather).

Device layout notes (per core):
  xt   [D, T] bf16 = x[b].T  -> projections produce qT/kT/vT [n, T] with head-dim on
       partitions, which feeds QK^T directly (scores transposed: [j, i], softmax sums
       over partitions via ones-matmul, PV uses v natural as lhsT).
  All matmuls bf16 with f32 PSUM accumulation. Softmax without max-subtraction:
  |scores| <= sqrt(128) by Cauchy-Schwarz after rmsnorm, so exp is safe in f32.
"""

import sys
from contextlib import ExitStack

for _p in ("/opt/trn_rl_repo", "/opt/pypackages"):
    if _p not in sys.path:
        sys.path.insert(0, _p)

import numpy as np
import ml_dtypes

import concourse.bass as bass
import concourse.mybir as mybir
import concourse.tile as tile
from concourse import bacc
from concourse.bass_utils import run_bass_kernel_spmd

bf16 = ml_dtypes.bfloat16
BF = mybir.dt.bfloat16
F32 = mybir.dt.float32
AF = mybir.ActivationFunctionType

B, T, D = 2, 2048, 2048
HD, H, G = 128, 4, 4
KC = D // 128          # 16 contraction chunks
TB = T // 128          # 16 t blocks
IC = T // 512          # 4 i chunks
EPS = 1e-6
MULT2 = float(HD) ** -0.5   # mult^2 folded into q gains

_NC_CACHE = {}


def _bcast(ap, p=128):
    """Partition-broadcast AP: [1, N] row -> [p, N] (step-0 partition dim)."""
    return bass.AP(tensor=ap.tensor, offset=ap.offset, ap=[[0, p]] + [list(a) for a in ap.ap[1:]])


def _build_nc():
    nc = bacc.Bacc(None)

    xt_d = nc.declare_dram_parameter("xt", [D, T], BF, isOutput=False)
    wq_d = nc.declare_dram_parameter("wq", [D, H * HD], BF, isOutput=False)
    wk_d = nc.declare_dram_parameter("wk", [D, HD], BF, isOutput=False)
    wv_d = nc.declare_dram_parameter("wv", [D, HD], BF, isOutput=False)
    wo_d = nc.declare_dram_parameter("wo", [H * HD, D], BF, isOutput=False)
    gqs_d = nc.declare_dram_parameter("gqs", [HD, H], F32, isOutput=False)
    gks_d = nc.declare_dram_parameter("gks", [HD, 1], F32, isOutput=False)
    cos_d = nc.declare_dram_parameter("cosf", [HD, T], BF, isOutput=False)
    sin_d = nc.declare_dram_parameter("sins", [HD, T], BF, isOutput=False)
    msk_d = nc.declare_dram_parameter("mask", [128, 128], F32, isOutput=False)
    idn_d = nc.declare_dram_parameter("ident", [128, 128], BF, isOutput=False)
    out_d = nc.declare_dram_parameter("out", [T, D], F32, isOutput=True)

    with tile.TileContext(nc) as tc:
        with ExitStack() as outer:
            persist = outer.enter_context(tc.tile_pool(name="persist", bufs=1))
            qhat = [persist.tile([128, T], BF, tag=f"qhat{h}", name=f"qhat{h}") for h in range(H)]
            khat = persist.tile([128, T], BF, tag="khat", name="khat")
            vnat = persist.tile([128, T], BF, tag="vnat", name="vnat")  # [j-local, tb*128+d]
            gqs = persist.tile([HD, H], F32, tag="gqs", name="gqs")
            gks = persist.tile([HD, 1], F32, tag="gks", name="gks")
            ones_bf = persist.tile([128, 1], BF, tag="ones", name="ones")
            eps_t = persist.tile([1, 1], F32, tag="eps", name="eps")

            nc.sync.dma_start(out=gqs, in_=gqs_d[:, :])
            nc.sync.dma_start(out=gks, in_=gks_d[:, :])
            nc.vector.memset(ones_bf, 1.0)
            nc.vector.memset(eps_t, EPS)

            # ---------------- Phase 1: projections + rmsnorm + rope ----------------
            with ExitStack() as s1:
                xt_p = s1.enter_context(tc.tile_pool(name="xt", bufs=1))
                w_p = s1.enter_context(tc.tile_pool(name="w", bufs=1))
                tmp_p = s1.enter_context(tc.tile_pool(name="tmp", bufs=1))
                row_p = s1.enter_context(tc.tile_pool(name="rows", bufs=1))
                dram_p = s1.enter_context(tc.tile_pool(name="dramb", bufs=2, space="DRAM"))
                ps_proj = s1.enter_context(tc.tile_pool(name="ps_proj", bufs=5, space="PSUM"))
                ps_row = s1.enter_context(tc.tile_pool(name="ps_row", bufs=2, space="PSUM"))
                ps_tp = s1.enter_context(tc.tile_pool(name="ps_tp", bufs=1, space="PSUM"))

                xt = []
                for kc in range(KC):
                    t_ = xt_p.tile([128, T], BF, tag=f"xt{kc}", name=f"xt{kc}")
                    nc.sync.dma_start(out=t_, in_=xt_d[kc * 128:(kc + 1) * 128, :])
                    xt.append(t_)
                wq_t, wk_t, wv_t = [], [], []
                for kc in range(KC):
                    a = w_p.tile([128, H * HD], BF, tag=f"wq{kc}", name=f"wq{kc}")
                    nc.sync.dma_start(out=a, in_=wq_d[kc * 128:(kc + 1) * 128, :])
                    wq_t.append(a)
                    b_ = w_p.tile([128, HD], BF, tag=f"wk{kc}", name=f"wk{kc}")
                    nc.sync.dma_start(out=b_, in_=wk_d[kc * 128:(kc + 1) * 128, :])
                    wk_t.append(b_)
                    c = w_p.tile([128, HD], BF, tag=f"wv{kc}", name=f"wv{kc}")
                    nc.sync.dma_start(out=c, in_=wv_d[kc * 128:(kc + 1) * 128, :])
                    wv_t.append(c)
                cosf = w_p.tile([HD, T], BF, tag="cosf", name="cosf")
                sins = w_p.tile([HD, T], BF, tag="sins", name="sins")
                ident = w_p.tile([128, 128], BF, tag="ident", name="ident")
                nc.sync.dma_start(out=cosf, in_=cos_d[:, :])
                nc.sync.dma_start(out=sins, in_=sin_d[:, :])
                nc.sync.dma_start(out=ident, in_=idn_d[:, :])

                def project(lhsT_of, raw_out):
                    """raw_out[:, tf*512:+512] = sum_kc lhsT(kc).T @ xt[kc][:, tf].
                    kc-outer with 4 live psums so consecutive matmuls share lhsT."""
                    pss = [ps_proj.tile([128, 512], F32, tag="ps_proj", name=f"ps_proj{tf}")
                           for tf in range(4)]
                    for kc in range(KC):
                        for tf in range(4):
                            nc.tensor.matmul(pss[tf], lhsT_of(kc),
                                             xt[kc][:, tf * 512:(tf + 1) * 512],
                                             start=(kc == 0), stop=(kc == KC - 1))
                    for tf in range(4):
                        nc.vector.tensor_copy(out=raw_out[:, tf * 512:(tf + 1) * 512],
                                              in_=pss[tf])

                def norm_rope(raw, gain_col, hat_out):
                    """hat_out (bf16) = rope(gain*raw) * rsqrt(mean(raw^2)+eps), per column t."""
                    sq = tmp_p.tile([128, T], BF, tag="sq", name="sq", bufs=2)
                    nc.vector.tensor_mul(sq, raw, raw)
                    srow = row_p.tile([1, T], F32, tag="srow", name="srow", bufs=1)
                    for tf in range(4):
                        pr = ps_row.tile([1, 512], F32, tag="ps_row", name="ps_row")
                        nc.tensor.matmul(pr, ones_bf, sq[:, tf * 512:(tf + 1) * 512],
                                         start=True, stop=True)
                        nc.vector.tensor_copy(out=srow[:, tf * 512:(tf + 1) * 512], in_=pr)
                    # srow = sqrt(srow/HD + eps); then in-place rinv = 1/srow
                    nc.scalar.activation(out=srow, in_=srow, func=AF.Sqrt,
                                         bias=eps_t[:, 0:1], scale=1.0 / HD)
                    nc.vector.reciprocal_approx_fast(out=srow, in_=srow)
                    srow_d = dram_p.tile([1, T], F32, tag="srow_d", name="srow_d")
                    nc.sync.dma_start(out=srow_d, in_=srow)
                    rb = tmp_p.tile([128, T], F32, tag="rb", name="rb", bufs=2)
                    nc.sync.dma_start(out=rb, in_=_bcast(srow_d))
                    # gain (per-partition scalar, bf16 out), then rotate-half rope in bf16
                    gt = tmp_p.tile([128, T], BF, tag="gt", name="gt", bufs=2)
                    nc.vector.tensor_scalar_mul(gt, raw, gain_col)
                    sw = tmp_p.tile([128, T], BF, tag="sw", name="sw", bufs=2)
                    nc.sync.dma_start(out=sw[0:64, :], in_=gt[64:128, :])
                    nc.sync.dma_start(out=sw[64:128, :], in_=gt[0:64, :])
                    t1 = tmp_p.tile([128, T], BF, tag="t1", name="t1", bufs=2)
                    nc.vector.tensor_mul(t1, gt, cosf)
                    nc.vector.tensor_mul(sw, sw, sins)
                    nc.vector.tensor_add(t1, t1, sw)
                    nc.vector.tensor_mul(hat_out, t1, rb)

                # K
                kraw = tmp_p.tile([128, T], F32, tag="raw", name="kraw", bufs=2)
                project(lambda kc: wk_t[kc], kraw)
                norm_rope(kraw, gks[:, 0:1], khat)
                # V (vT then 128x128 transposes to natural layout)
                vtr = tmp_p.tile([128, T], BF, tag="sq", name="vtr", bufs=2)
                project(lambda kc: wv_t[kc], vtr)
                for tb in range(TB):
                    pt = ps_tp.tile([128, 128], BF, tag="ps_tp", name="ps_tp")
                    nc.tensor.transpose(pt, vtr[:, tb * 128:(tb + 1) * 128], ident)
                    nc.vector.tensor_copy(out=vnat[:, tb * 128:(tb + 1) * 128], in_=pt)
                # Q heads
                for h in range(H):
                    qraw = tmp_p.tile([128, T], F32, tag="raw", name=f"qraw{h}", bufs=2)
                    project(lambda kc: wq_t[kc][:, h * 128:(h + 1) * 128], qraw)
                    norm_rope(qraw, gqs[:, h:h + 1], qhat[h])

            # ------- Phases 2+3: causal attention interleaved with output projection ---
            # ic-outer: after each 512-wide query chunk finishes attention for all
            # heads, its 4 t-blocks run the output projection + DMA immediately, so
            # the 16.8MB output write spreads across the kernel instead of piling at
            # the tail.
            with ExitStack() as s2:
                o_p = s2.enter_context(tc.tile_pool(name="op", bufs=1))
                oT = [o_p.tile([128, T], BF, tag=f"oT{h}", name=f"oT{h}") for h in range(H)]
                wo_p = s2.enter_context(tc.tile_pool(name="wo", bufs=1))
                mask_p = s2.enter_context(tc.tile_pool(name="maskp", bufs=1))
                p_p = s2.enter_context(tc.tile_pool(name="pexp", bufs=72))
                dn_p = s2.enter_context(tc.tile_pool(name="dn", bufs=2))
                db_p = s2.enter_context(tc.tile_pool(name="dnb", bufs=2))
                ost_p = s2.enter_context(tc.tile_pool(name="ost", bufs=6))
                dram2_p = s2.enter_context(tc.tile_pool(name="dramb2", bufs=4, space="DRAM"))
                ps_acc = s2.enter_context(tc.tile_pool(name="ps_acc", bufs=4, space="PSUM"))
                ps_s = s2.enter_context(tc.tile_pool(name="ps_s", bufs=4, space="PSUM"))

                wo_t = []
                for h in range(H):
                    w_ = wo_p.tile([128, D], BF, tag=f"wo{h}", name=f"wo{h}")
                    nc.sync.dma_start(out=w_, in_=wo_d[h * 128:(h + 1) * 128, :])
                    wo_t.append(w_)
                mask = mask_p.tile([128, 128], F32, tag="mask", name="mask")
                nc.sync.dma_start(out=mask, in_=msk_d[:, :])

                for ic in range(IC):
                    i0 = ic * 512
                    jb_max = 4 * ic + 3
                    po = [ps_acc.tile([128, 512], F32, tag="acc", name=f"po{h}")
                          for h in range(H)]
                    offs = [max(0, 128 * (jb - 4 * ic)) for jb in range(jb_max + 1)]
                    pt = {}  # (h, jb) -> exp tile, consumed by PV now + dn chain later
                    for jb in range(jb_max + 1):
                        off = offs[jb]
                        kb = khat[:, jb * 128:(jb + 1) * 128]
                        vb = vnat[:, jb * 128:(jb + 1) * 128]
                        pss = []
                        for h in range(H):  # 4 QKs share the stationary khat block
                            ps = ps_s.tile([128, 512], F32, tag="sc", name="sc")
                            nc.tensor.matmul(ps[:, off:], kb,
                                             qhat[h][:, i0 + off:i0 + 512],
                                             start=True, stop=True)
                            if jb >= 4 * ic:
                                nc.vector.tensor_add(ps[:, off:off + 128],
                                                     ps[:, off:off + 128], mask)
                            pss.append(ps)
                        for h in range(H):
                            p = p_p.tile([128, 512], BF, tag="p", name="p")
                            nc.scalar.activation(out=p[:, off:], in_=pss[h][:, off:],
                                                 func=AF.Exp)
                            pt[(h, jb)] = p
                        for h in range(H):  # 4 PVs share the stationary vnat block
                            nc.tensor.matmul(po[h][:, off:], vb, pt[(h, jb)][:, off:],
                                             start=(jb == 0), stop=(jb == jb_max))
                    for h in range(H):
                        # denominator: ones-matmul chain over this head's exp tiles
                        pdn = ps_s.tile([1, 512], F32, tag="sc", name="pdn")
                        for jb in range(jb_max + 1):
                            nc.tensor.matmul(pdn[:, offs[jb]:], ones_bf,
                                             pt[(h, jb)][:, offs[jb]:],
                                             start=(jb == 0), stop=(jb == jb_max))
                        drow = dn_p.tile([1, 512], F32, tag="drow", name="drow")
                        nc.vector.tensor_copy(out=drow, in_=pdn)
                        nc.vector.reciprocal_approx_fast(out=drow, in_=drow)
                        drow_d = dram2_p.tile([1, 512], F32, tag="drow_d", name="drow_d")
                        nc.sync.dma_start(out=drow_d, in_=drow)
                        db = db_p.tile([128, 512], F32, tag="db", name="db")
                        nc.sync.dma_start(out=db, in_=_bcast(drow_d))
                        nc.vector.tensor_mul(oT[h][:, i0:i0 + 512], po[h], db)
                    # output projection for this chunk's 4 t-blocks; oc-inner over 4
                    # live psums so each oT block's LDWEIGHTS is shared 4 ways
                    for tb in range(4 * ic, 4 * ic + 4):
                        psos = [ps_acc.tile([128, 512], F32, tag="acc", name=f"pso{oc}")
                                for oc in range(4)]
                        for h in range(H):
                            for oc in range(4):
                                nc.tensor.matmul(psos[oc], oT[h][:, tb * 128:(tb + 1) * 128],
                                                 wo_t[h][:, oc * 512:(oc + 1) * 512],
                                                 start=(h == 0), stop=(h == H - 1))
                        for oc in range(4):
                            ost = ost_p.tile([128, 512], F32, tag="ost", name="ost")
                            nc.vector.tensor_copy(out=ost, in_=psos[oc])
                            nc.sync.dma_start(out=out_d[tb * 128:(tb + 1) * 128,
                                                        oc * 512:(oc + 1) * 512], in_=ost)
    nc.finalize()
    return nc


def _rope_tables():
    d = np.arange(64, dtype=np.float64)
    ang = 10000.0 ** (-d / 64.0)
    pos = np.arange(T, dtype=np.float64)
    rad = pos[None, :] * ang[:, None]          # [64, T]
    cos, sin = np.cos(rad), np.sin(rad)
    cosF = np.concatenate([cos, cos], 0).astype(bf16)
    sinS = np.concatenate([-sin, sin], 0).astype(bf16)
    return np.ascontiguousarray(cosF), np.ascontiguousarray(sinS)


def _in_maps(x, wq, wk, wv, wo, gq, gk):
    cosF, sinS = _rope_tables()
    mask = np.ascontiguousarray(np.triu(np.full((128, 128), -1e9, np.float32), 1).T)
    ident = np.eye(128, dtype=bf16)
    maps = []
    for core in range(8):
        b, g = core // 4, core % 4
        maps.append({
            "xt": np.ascontiguousarray(x[b].T).astype(bf16),
            "wq": np.ascontiguousarray(wq[:, g * 512:(g + 1) * 512]).astype(bf16),
            "wk": np.ascontiguousarray(wk[:, g * 128:(g + 1) * 128]).astype(bf16),
            "wv": np.ascontiguousarray(wv[:, g * 128:(g + 1) * 128]).astype(bf16),
            "wo": np.ascontiguousarray(wo[g * 512:(g + 1) * 512, :]).astype(bf16),
            "gqs": np.ascontiguousarray((gq[g].T * MULT2).astype(np.float32)),
            "gks": np.ascontiguousarray(gk[g].astype(np.float32).reshape(HD, 1)),
            "cosf": cosF, "sins": sinS, "mask": mask, "ident": ident,
        })
    return maps


def _get_nc():
    if "nc" not in _NC_CACHE:
        _NC_CACHE["nc"] = _build_nc()
    return _NC_CACHE["nc"]


def _run(inputs, trace=False, trace_kwargs=None, tmpdir=None):
    nc = _get_nc()
    maps = _in_maps(inputs["x"], inputs["wq"], inputs["wk"], inputs["wv"],
                    inputs["wo"], inputs["gq"], inputs["gk"])
    res = run_bass_kernel_spmd(nc, maps, core_ids=list(range(8)), trace=trace,
                               tmpdir=tmpdir, **(trace_kwargs or {}))
    out = np.zeros((B, T, D), np.float32)
    for core in range(8):
        out[core // 4] += res.results[core]["out"]
    return out, res


def kernel(**inputs):
    inputs = {k: np.asarray(v) for k, v in inputs.items()}
    out, _ = _run(inputs, trace=False)
    return out


# revision 24
# speedup vs baseline: 1.2272x; 1.0717x over previous
"""Grouped-query attention (B=2,T=2048,D=2048, 4 groups x 4 heads x 128d) on 8 trn2 cores.

Sharding: core = (batch b, group g); b = core//4, g = core%4 (data parallel x tensor
parallel). Each core computes its group's QKV projections, QK-rmsnorm+rope, causal
flash-style attention, and a partial output projection o_g @ wo_g; the host sums the
4 per-group partials per batch (row-sharded wo all-reduce done on host at gather).

Device layout notes (per core):
  xt   [D, T] bf16 = x[b].T  -> projections produce qT/kT/vT [n, T] with head-dim on
       partitions, which feeds QK^T directly (scores transposed: [j, i], softmax sums
       over partitions via ones-matmul, PV uses v natural as lhsT).
  All matmuls bf16 with f32 PSUM accumulation. Softmax without max-subtraction:
  |scores| <= sqrt(128) by Cauchy-Schwarz after rmsnorm, so exp is safe in f32.
"""

import sys
from contextlib import ExitStack

for _p in ("/opt/trn_rl_repo", "/opt/pypackages"):
    if _p not in sys.path:
        sys.path.insert(0, _p)

import numpy as np
import ml_dtypes

import concourse.bass as bass
import concourse.mybir as mybir
import concourse.tile as tile
from concourse import bacc
from concourse.bass_utils import run_bass_kernel_spmd

bf16 = ml_dtypes.bfloat16
BF = mybir.dt.bfloat16
F32 = mybir.dt.float32
AF = mybir.ActivationFunctionType

B, T, D = 2, 2048, 2048
HD, H, G = 128, 4, 4
KC = D // 128          # 16 contraction chunks
TB = T // 128          # 16 t blocks
IC = T // 512          # 4 i chunks
EPS = 1e-6
MULT2 = float(HD) ** -0.5   # mult^2 folded into q gains

_NC_CACHE = {}


def _bcast(ap, p=128):
    """Partition-broadcast AP: [1, N] row -> [p, N] (step-0 partition dim)."""
    return bass.AP(tensor=ap.tensor, offset=ap.offset, ap=[[0, p]] + [list(a) for a in ap.ap[1:]])


def _build_nc():
    nc = bacc.Bacc(None)

    xt_d = nc.declare_dram_parameter("xt", [D, T], BF, isOutput=False)
    wq_d = nc.declare_dram_parameter("wq", [D, H * HD], BF, isOutput=False)
    wk_d = nc.declare_dram_parameter("wk", [D, HD], BF, isOutput=False)
    wv_d = nc.declare_dram_parameter("wv", [D, HD], BF, isOutput=False)
    wo_d = nc.declare_dram_parameter("wo", [H * HD, D], BF, isOutput=False)
    gqs_d = nc.declare_dram_parameter("gqs", [HD, H], F32, isOutput=False)
    gks_d = nc.declare_dram_parameter("gks", [HD, 1], F32, isOutput=False)
    cos_d = nc.declare_dram_parameter("cosf", [HD, T], BF, isOutput=False)
    sin_d = nc.declare_dram_parameter("sins", [HD, T], BF, isOutput=False)
    msk_d = nc.declare_dram_parameter("mask", [128, 128], F32, isOutput=False)
    idn_d = nc.declare_dram_parameter("ident", [128, 128], BF, isOutput=False)
    out_d = nc.declare_dram_parameter("out", [T, D], F32, isOutput=True)

    with tile.TileContext(nc) as tc:
        with ExitStack() as outer:
            persist = outer.enter_context(tc.tile_pool(name="persist", bufs=1))
            qhat = [persist.tile([128, T], BF, tag=f"qhat{h}", name=f"qhat{h}") for h in range(H)]
            khat = persist.tile([128, T], BF, tag="khat", name="khat")
            vnat = persist.tile([128, T], BF, tag="vnat", name="vnat")  # [j-local, tb*128+d]
            gqs = persist.tile([HD, H], F32, tag="gqs", name="gqs")
            gks = persist.tile([HD, 1], F32, tag="gks", name="gks")
            ones_bf = persist.tile([128, 1], BF, tag="ones", name="ones")
            eps_t = persist.tile([1, 1], F32, tag="eps", name="eps")

            nc.sync.dma_start(out=gqs, in_=gqs_d[:, :])
            nc.sync.dma_start(out=gks, in_=gks_d[:, :])
            nc.vector.memset(ones_bf, 1.0)
            nc.vector.memset(eps_t, EPS)

            # ---------------- Phase 1: projections + rmsnorm + rope ----------------
            with ExitStack() as s1:
                xt_p = s1.enter_context(tc.tile_pool(name="xt", bufs=1))
                w_p = s1.enter_context(tc.tile_pool(name="w", bufs=1))
                tmp_p = s1.enter_context(tc.tile_pool(name="tmp", bufs=1))
                row_p = s1.enter_context(tc.tile_pool(name="rows", bufs=1))
                dram_p = s1.enter_context(tc.tile_pool(name="dramb", bufs=2, space="DRAM"))
                ps_proj = s1.enter_context(tc.tile_pool(name="ps_proj", bufs=5, space="PSUM"))
                ps_row = s1.enter_context(tc.tile_pool(name="ps_row", bufs=2, space="PSUM"))
                ps_tp = s1.enter_context(tc.tile_pool(name="ps_tp", bufs=1, space="PSUM"))

                # DMA order matters: interleave weights with xt chunks so the first
                # projection chain (K) can start after ~2 chunks instead of waiting
                # behind the full 8.4MB of xt.
                xt, wq_t, wk_t, wv_t = [], [], [], []
                for kc in range(KC):
                    b_ = w_p.tile([128, HD], BF, tag=f"wk{kc}", name=f"wk{kc}")
                    nc.sync.dma_start(out=b_, in_=wk_d[kc * 128:(kc + 1) * 128, :])
                    wk_t.append(b_)
                    t_ = xt_p.tile([128, T], BF, tag=f"xt{kc}", name=f"xt{kc}")
                    nc.sync.dma_start(out=t_, in_=xt_d[kc * 128:(kc + 1) * 128, :])
                    xt.append(t_)
                for kc in range(KC):
                    c = w_p.tile([128, HD], BF, tag=f"wv{kc}", name=f"wv{kc}")
                    nc.sync.dma_start(out=c, in_=wv_d[kc * 128:(kc + 1) * 128, :])
                    wv_t.append(c)
                for kc in range(KC):
                    a = w_p.tile([128, H * HD], BF, tag=f"wq{kc}", name=f"wq{kc}")
                    nc.sync.dma_start(out=a, in_=wq_d[kc * 128:(kc + 1) * 128, :])
                    wq_t.append(a)
                cosf = w_p.tile([HD, T], BF, tag="cosf", name="cosf")
                sins = w_p.tile([HD, T], BF, tag="sins", name="sins")
                ident = w_p.tile([128, 128], BF, tag="ident", name="ident")
                nc.sync.dma_start(out=cosf, in_=cos_d[:, :])
                nc.sync.dma_start(out=sins, in_=sin_d[:, :])
                nc.sync.dma_start(out=ident, in_=idn_d[:, :])

                def project(lhsT_of, raw_out):
                    """raw_out[:, tf*512:+512] = sum_kc lhsT(kc).T @ xt[kc][:, tf].
                    kc-outer with 4 live psums so consecutive matmuls share lhsT."""
                    pss = [ps_proj.tile([128, 512], F32, tag="ps_proj", name=f"ps_proj{tf}")
                           for tf in range(4)]
                    for kc in range(KC):
                        for tf in range(4):
                            nc.tensor.matmul(pss[tf], lhsT_of(kc),
                                             xt[kc][:, tf * 512:(tf + 1) * 512],
                                             start=(kc == 0), stop=(kc == KC - 1))
                    for tf in range(4):
                        nc.vector.tensor_copy(out=raw_out[:, tf * 512:(tf + 1) * 512],
                                              in_=pss[tf])

                def norm_rope(raw, gain_col, hat_out):
                    """hat_out (bf16) = rope(gain*raw) * rsqrt(mean(raw^2)+eps), per column t."""
                    sq = tmp_p.tile([128, T], BF, tag="sq", name="sq", bufs=2)
                    nc.scalar.square(out=sq, in_=raw)
                    srow = row_p.tile([1, T], F32, tag="srow", name="srow", bufs=1)
                    for tf in range(4):
                        pr = ps_row.tile([1, 512], F32, tag="ps_row", name="ps_row")
                        nc.tensor.matmul(pr, ones_bf, sq[:, tf * 512:(tf + 1) * 512],
                                         start=True, stop=True)
                        nc.vector.tensor_copy(out=srow[:, tf * 512:(tf + 1) * 512], in_=pr)
                    # srow = sqrt(srow/HD + eps); then in-place rinv = 1/srow
                    nc.scalar.activation(out=srow, in_=srow, func=AF.Sqrt,
                                         bias=eps_t[:, 0:1], scale=1.0 / HD)
                    nc.vector.reciprocal_approx_fast(out=srow, in_=srow)
                    srow_d = dram_p.tile([1, T], F32, tag="srow_d", name="srow_d")
                    nc.sync.dma_start(out=srow_d, in_=srow)
                    rb = tmp_p.tile([128, T], F32, tag="rb", name="rb", bufs=2)
                    nc.sync.dma_start(out=rb, in_=_bcast(srow_d))
                    # gain (per-partition scalar, bf16 out), then rotate-half rope in bf16
                    gt = tmp_p.tile([128, T], BF, tag="gt", name="gt", bufs=2)
                    nc.vector.tensor_scalar_mul(gt, raw, gain_col)
                    sw = tmp_p.tile([128, T], BF, tag="sw", name="sw", bufs=2)
                    nc.sync.dma_start(out=sw[0:64, :], in_=gt[64:128, :])
                    nc.sync.dma_start(out=sw[64:128, :], in_=gt[0:64, :])
                    t1 = tmp_p.tile([128, T], BF, tag="t1", name="t1", bufs=2)
                    nc.vector.tensor_mul(t1, gt, cosf)
                    nc.gpsimd.tensor_mul(sw, sw, sins)
                    nc.vector.tensor_add(t1, t1, sw)
                    nc.vector.tensor_mul(hat_out, t1, rb)

                # K
                kraw = tmp_p.tile([128, T], F32, tag="raw", name="kraw", bufs=2)
                project(lambda kc: wk_t[kc], kraw)
                norm_rope(kraw, gks[:, 0:1], khat)
                # V (vT then 128x128 transposes to natural layout)
                vtr = tmp_p.tile([128, T], BF, tag="sq", name="vtr", bufs=2)
                project(lambda kc: wv_t[kc], vtr)
                for tb in range(TB):
                    pt = ps_tp.tile([128, 128], BF, tag="ps_tp", name="ps_tp")
                    nc.tensor.transpose(pt, vtr[:, tb * 128:(tb + 1) * 128], ident)
                    nc.vector.tensor_copy(out=vnat[:, tb * 128:(tb + 1) * 128], in_=pt)
                # Q heads
                for h in range(H):
                    qraw = tmp_p.tile([128, T], F32, tag="raw", name=f"qraw{h}", bufs=2)
                    project(lambda kc: wq_t[kc][:, h * 128:(h + 1) * 128], qraw)
                    norm_rope(qraw, gqs[:, h:h + 1], qhat[h])

            # ------- Phases 2+3: causal attention interleaved with output projection ---
            # ic-outer: after each 512-wide query chunk finishes attention for all
            # heads, its 4 t-blocks run the output projection + DMA immediately, so
            # the 16.8MB output write spreads across the kernel instead of piling at
            # the tail.
            with ExitStack() as s2:
                o_p = s2.enter_context(tc.tile_pool(name="op", bufs=1))
                oT = [o_p.tile([128, T], BF, tag=f"oT{h}", name=f"oT{h}") for h in range(H)]
                wo_p = s2.enter_context(tc.tile_pool(name="wo", bufs=1))
                mask_p = s2.enter_context(tc.tile_pool(name="maskp", bufs=1))
                p_p = s2.enter_context(tc.tile_pool(name="pexp", bufs=72))
                dn_p = s2.enter_context(tc.tile_pool(name="dn", bufs=2))
                db_p = s2.enter_context(tc.tile_pool(name="dnb", bufs=2))
                ost_p = s2.enter_context(tc.tile_pool(name="ost", bufs=6))
                dram2_p = s2.enter_context(tc.tile_pool(name="dramb2", bufs=4, space="DRAM"))
                ps_acc = s2.enter_context(tc.tile_pool(name="ps_acc", bufs=4, space="PSUM"))
                ps_s = s2.enter_context(tc.tile_pool(name="ps_s", bufs=4, space="PSUM"))

                wo_t = []
                for h in range(H):
                    w_ = wo_p.tile([128, D], BF, tag=f"wo{h}", name=f"wo{h}")
                    nc.sync.dma_start(out=w_, in_=wo_d[h * 128:(h + 1) * 128, :])
                    wo_t.append(w_)
                mask = mask_p.tile([128, 128], F32, tag="mask", name="mask")
                nc.sync.dma_start(out=mask, in_=msk_d[:, :])

                for ic in range(IC):
                    i0 = ic * 512
                    jb_max = 4 * ic + 3
                    po = [ps_acc.tile([128, 512], F32, tag="acc", name=f"po{h}")
                          for h in range(H)]
                    offs = [max(0, 128 * (jb - 4 * ic)) for jb in range(jb_max + 1)]
                    pt = {}  # (h, jb) -> exp tile, consumed by PV now + dn chain later
                    for jb in range(jb_max + 1):
                        off = offs[jb]
                        kb = khat[:, jb * 128:(jb + 1) * 128]
                        vb = vnat[:, jb * 128:(jb + 1) * 128]
                        pss = []
                        for h in range(H):  # 4 QKs share the stationary khat block
                            ps = ps_s.tile([128, 512], F32, tag="sc", name="sc")
                            nc.tensor.matmul(ps[:, off:], kb,
                                             qhat[h][:, i0 + off:i0 + 512],
                                             start=True, stop=True)
                            if jb >= 4 * ic:
                                nc.vector.tensor_add(ps[:, off:off + 128],
                                                     ps[:, off:off + 128], mask)
                            pss.append(ps)
                        for h in range(H):
                            p = p_p.tile([128, 512], BF, tag="p", name="p")
                            nc.scalar.activation(out=p[:, off:], in_=pss[h][:, off:],
                                                 func=AF.Exp)
                            pt[(h, jb)] = p
                        for h in range(H):  # 4 PVs share the stationary vnat block
                            nc.tensor.matmul(po[h][:, off:], vb, pt[(h, jb)][:, off:],
                                             start=(jb == 0), stop=(jb == jb_max))
                    for h in range(H):
                        # denominator: ones-matmul chain over this head's exp tiles
                        pdn = ps_s.tile([1, 512], F32, tag="sc", name="pdn")
                        for jb in range(jb_max + 1):
                            nc.tensor.matmul(pdn[:, offs[jb]:], ones_bf,
                                             pt[(h, jb)][:, offs[jb]:],
                                             start=(jb == 0), stop=(jb == jb_max))
                        drow = dn_p.tile([1, 512], F32, tag="drow", name="drow")
                        nc.vector.tensor_copy(out=drow, in_=pdn)
                        nc.vector.reciprocal_approx_fast(out=drow, in_=drow)
                        drow_d = dram2_p.tile([1, 512], F32, tag="drow_d", name="drow_d")
                        nc.sync.dma_start(out=drow_d, in_=drow)
                        db = db_p.tile([128, 512], F32, tag="db", name="db")
                        nc.sync.dma_start(out=db, in_=_bcast(drow_d))
                        nc.vector.tensor_mul(oT[h][:, i0:i0 + 512], po[h], db)
                    # output projection for this chunk's 4 t-blocks; oc-inner over 4
                    # live psums so each oT block's LDWEIGHTS is shared 4 ways
                    for tb in range(4 * ic, 4 * ic + 4):
                        psos = [ps_acc.tile([128, 512], F32, tag="acc", name=f"pso{oc}")
                                for oc in range(4)]
                        for h in range(H):
                            for oc in range(4):
                                nc.tensor.matmul(psos[oc], oT[h][:, tb * 128:(tb + 1) * 128],
                                                 wo_t[h][:, oc * 512:(oc + 1) * 512],
                                                 start=(h == 0), stop=(h == H - 1))
                        for oc in range(4):
                            ost = ost_p.tile([128, 512], F32, tag="ost", name="ost")
                            nc.vector.tensor_copy(out=ost, in_=psos[oc])
                            nc.sync.dma_start(out=out_d[tb * 128:(tb + 1) * 128,
                                                        oc * 512:(oc + 1) * 512], in_=ost)
    nc.finalize()
    return nc


def _rope_tables():
    d = np.arange(64, dtype=np.float64)
    ang = 10000.0 ** (-d / 64.0)
    pos = np.arange(T, dtype=np.float64)
    rad = pos[None, :] * ang[:, None]          # [64, T]
    cos, sin = np.cos(rad), np.sin(rad)
    cosF = np.concatenate([cos, cos], 0).astype(bf16)
    sinS = np.concatenate([-sin, sin], 0).astype(bf16)
    return np.ascontiguousarray(cosF), np.ascontiguousarray(sinS)


def _in_maps(x, wq, wk, wv, wo, gq, gk):
    cosF, sinS = _rope_tables()
    mask = np.ascontiguousarray(np.triu(np.full((128, 128), -1e9, np.float32), 1).T)
    ident = np.eye(128, dtype=bf16)
    maps = []
    for core in range(8):
        b, g = core // 4, core % 4
        maps.append({
            "xt": np.ascontiguousarray(x[b].T).astype(bf16),
            "wq": np.ascontiguousarray(wq[:, g * 512:(g + 1) * 512]).astype(bf16),
            "wk": np.ascontiguousarray(wk[:, g * 128:(g + 1) * 128]).astype(bf16),
            "wv": np.ascontiguousarray(wv[:, g * 128:(g + 1) * 128]).astype(bf16),
            "wo": np.ascontiguousarray(wo[g * 512:(g + 1) * 512, :]).astype(bf16),
            "gqs": np.ascontiguousarray((gq[g].T * MULT2).astype(np.float32)),
            "gks": np.ascontiguousarray(gk[g].astype(np.float32).reshape(HD, 1)),
            "cosf": cosF, "sins": sinS, "mask": mask, "ident": ident,
        })
    return maps


def _get_nc():
    if "nc" not in _NC_CACHE:
        _NC_CACHE["nc"] = _build_nc()
    return _NC_CACHE["nc"]


def _run(inputs, trace=False, trace_kwargs=None, tmpdir=None):
    nc = _get_nc()
    maps = _in_maps(inputs["x"], inputs["wq"], inputs["wk"], inputs["wv"],
                    inputs["wo"], inputs["gq"], inputs["gk"])
    res = run_bass_kernel_spmd(nc, maps, core_ids=list(range(8)), trace=trace,
                               tmpdir=tmpdir, **(trace_kwargs or {}))
    out = np.zeros((B, T, D), np.float32)
    for core in range(8):
        out[core // 4] += res.results[core]["out"]
    return out, res


def kernel(**inputs):
    inputs = {k: np.asarray(v) for k, v in inputs.items()}
    out, _ = _run(inputs, trace=False)
    return out
